# revision 1
# baseline (speedup 1.0000x reference)
"""Trainium2 Bass kernel for nn_BSLoss (Black-Scholes PINN loss on a 4096x4096 grid).

Strategy (8 NeuronCores, SPMD):
  - Shard the [N_S, N_t] grid along S: 512 rows/core plus 1-row halos (host-sliced).
  - The PDE residual r = c_t*(V[:,j+1]-V[:,j-1]) + tridiag_S(V) is built entirely on
    the TensorEngine: one [128x128] tridiagonal stationary matmul (contraction over
    the partition axis gives the S-stencil cross-partition shifts for free) plus two
    +/- identity matmuls against column-shifted moving APs for the t-stencil, all
    accumulated in one PSUM bank (float32r = tf32 inputs, fp32 accumulate).
  - ScalarEngine activation(Square, scale=c, accum_out) squares and reduces each
    [128, 512] residual chunk to per-partition sums in one instruction.
  - Host applies row masks (grid boundary + halo rows) to the tiny [128, 40] stats,
    sums in float64, and computes the O(N) boundary losses (rows 0/4095, col 4095).

Grid boundary columns (t=0, t=4095) are excluded by construction: chunks only cover
columns 1..4094, so no column masking is needed on device.
"""
import sys

if "/opt/trn_rl_repo" not in sys.path:
    sys.path.insert(0, "/opt/trn_rl_repo")

import numpy as np

import concourse.mybir as mybir
import concourse.tile as tile
from concourse import bacc
from concourse.bass_utils import run_bass_kernel_spmd

# ---- problem constants (match the reference) ----
N_S, N_T = 4096, 4096
R, SIGMA, K, T_MAT, SMAX = 0.05, 0.2, 100.0, 1.0, 300.0
B_STR, ALPHA = K / SMAX, 0.5
L_PDE, L_BC, L_TC = 1.0, 10.0, 10.0
HUBER_DELTA = 0.01
SOFTPLUS_BETA = 50.0

N_CORES = 8
ROWS_PER_CORE = N_S // N_CORES          # 512
IN_ROWS = ROWS_PER_CORE + 2             # 514 (with halos)
P = 128
TILE_STARTS = [0, 126, 252, 378]        # full tiles; outputs local rows 1..504
STRIP_START = 504                       # strip tile rows 504..513 -> outputs 505..512
STRIP_K = 10
N_TILES = 5
CHUNKS = [(1 + 512 * i, 512) for i in range(7)] + [(3585, 510)]  # cover cols 1..4094
N_UNITS = N_TILES * len(CHUNKS)
C_T = (N_T - 1) / 2.0 / T_MAT           # 1/(2*dt_norm)/T_MAT = 2047.5

F32 = mybir.dt.float32
F32R = mybir.dt.float32r

# stationary blocks packed along free dim: [Wtri0..4, Ip_full, Im_full, Ip_strip, Im_strip]
N_WBLK = 9
WBLK = {"tri": lambda t: t, "ip_full": 5, "im_full": 6, "ip_strip": 7, "im_strip": 8}


def _solve_cubic(Q: float) -> float:
    c = -Q
    for _ in range(5):
        f = c ** 3 / 6.0 + c + Q
        df = 0.5 * c * c + 1.0
        c = c - f / df
    return c


C1 = _solve_cubic((B_STR - 0.0) / ALPHA)
C2 = _solve_cubic((B_STR - 1.0) / ALPHA)


def _tf32(x: np.ndarray) -> np.ndarray:
    """Round float32 to tfloat32 (10-bit mantissa, round-to-nearest)."""
    u = np.ascontiguousarray(x, dtype=np.float32).view(np.uint32).astype(np.uint64)
    u = (u + np.uint64(0x1000)) & np.uint64(0xFFFFE000)
    return u.astype(np.uint32).view(np.float32)


def _stencil_coeffs(S: np.ndarray):
    """Per-row coefficients of the PDE residual stencil, float64.

    residual = c_t*(V+ - V-)_t + hi(g)*V[g+1] + mid(g)*V[g] + lo(g)*V[g-1]
    The returned coefficients are divided by C_T (folded back via the ACT scale).
    """
    S = S.astype(np.float64)
    dS = 1.0 / (N_S - 1)
    L = C2 * S + C1 * (1.0 - S)
    dL = C2 - C1
    S_u = ALPHA * dL * (0.5 * L ** 2 + 1.0)
    S_uu = ALPHA * dL ** 2 * L
    e = 0.5 * SIGMA ** 2 * S ** 2
    f = R * S
    a_uu = e / S_u ** 2
    a_u = f / S_u - e * S_uu / S_u ** 3
    hi = a_uu / dS ** 2 + a_u / (2 * dS)
    lo = a_uu / dS ** 2 - a_u / (2 * dS)
    mid = -2.0 * a_uu / dS ** 2 - R
    return lo / C_T, mid / C_T, hi / C_T


_PROGRAM = None


def _build_program():
    """Build the SPMD Bass program (identical for all cores)."""
    nc = bacc.Bacc("TRN2", target_bir_lowering=False)

    v_in = nc.dram_tensor("v_in", [IN_ROWS, N_T], F32R, kind="ExternalInput")
    w_in = nc.dram_tensor("w_in", [P, N_WBLK * P], F32R, kind="ExternalInput")
    stats_out = nc.dram_tensor("stats_out", [P, N_UNITS], F32, kind="ExternalOutput")

    with tile.TileContext(nc) as tc:
        with (
            tc.tile_pool(name="vpool", bufs=1) as vpool,
            tc.tile_pool(name="wpool", bufs=1) as wpool,
            tc.tile_pool(name="spool", bufs=1) as spool,
            tc.tile_pool(name="psum", bufs=6, space="PSUM") as psum_pool,
        ):
            w = wpool.tile([P, N_WBLK * P], F32R)
            nc.sync.dma_start(w[:], w_in[:])

            vtiles = []
            for t in range(N_TILES):
                if t < 4:
                    vt = vpool.tile([P, N_T], F32R, tag=f"vt{t}")
                    nc.sync.dma_start(vt[:], v_in[TILE_STARTS[t]:TILE_STARTS[t] + P, :])
                else:
                    vt = vpool.tile([STRIP_K, N_T], F32R, tag="vstrip")
                    nc.sync.dma_start(vt[:], v_in[STRIP_START:STRIP_START + STRIP_K, :])
                vtiles.append(vt)

            stats = spool.tile([P, N_UNITS], F32)

            def wblk(idx, kdim):
                return w[0:kdim, idx * P:(idx + 1) * P]

            for t in range(N_TILES):
                kdim = P if t < 4 else STRIP_K
                vt = vtiles[t]
                tri = wblk(WBLK["tri"](t), kdim)
                ip = wblk(WBLK["ip_full"] if t < 4 else WBLK["ip_strip"], kdim)
                im = wblk(WBLK["im_full"] if t < 4 else WBLK["im_strip"], kdim)
                for ci, (a, cw) in enumerate(CHUNKS):
                    u = t * len(CHUNKS) + ci
                    ps = psum_pool.tile([P, cw], F32, tag="ps")
                    nc.tensor.matmul(ps[:], lhsT=tri, rhs=vt[0:kdim, a:a + cw],
                                     start=True, stop=False)
                    nc.tensor.matmul(ps[:], lhsT=ip, rhs=vt[0:kdim, a + 1:a + 1 + cw],
                                     start=False, stop=False)
                    nc.tensor.matmul(ps[:], lhsT=im, rhs=vt[0:kdim, a - 1:a - 1 + cw],
                                     start=False, stop=True)
                    nc.scalar.activation(ps[:], ps[:],
                                         mybir.ActivationFunctionType.Square,
                                         scale=float(C_T),
                                         accum_out=stats[:, u:u + 1])

            nc.sync.dma_start(stats_out[:], stats[:])

    nc.compile()
    return nc


def _host_inputs_and_masks(V: np.ndarray, S: np.ndarray):
    """Per-core in_maps plus the [core][P, N_UNITS] row masks."""
    lo, mid, hi = _stencil_coeffs(S)

    in_maps = []
    masks = []
    for c in range(N_CORES):
        rows = np.clip(np.arange(512 * c - 1, 512 * c + 513), 0, N_S - 1)
        v_shard = _tf32(V[rows, :])

        wbuf = np.zeros((P, N_WBLK * P), np.float64)
        mask = np.zeros((P, N_UNITS), np.float32)
        for t in range(N_TILES):
            if t < 4:
                t0, kdim, m_lo, m_hi = TILE_STARTS[t], P, 1, 126
            else:
                t0, kdim, m_lo, m_hi = STRIP_START, STRIP_K, 1, 8
            blk = WBLK["tri"](t)
            for m in range(m_lo, m_hi + 1):
                L = t0 + m                      # local output row (1..512)
                g = 512 * c - 1 + L             # global grid row
                if not (1 <= g <= N_S - 2):
                    continue
                wbuf[m - 1, blk * P + m] = lo[g]
                wbuf[m, blk * P + m] = mid[g]
                wbuf[m + 1, blk * P + m] = hi[g]
                mask[m, t * len(CHUNKS):(t + 1) * len(CHUNKS)] = 1.0
        for m in range(P):                      # identity blocks (exact +/-1)
            wbuf[m, WBLK["ip_full"] * P + m] = 1.0
            wbuf[m, WBLK["im_full"] * P + m] = -1.0
        for m in range(STRIP_K):
            wbuf[m, WBLK["ip_strip"] * P + m] = 1.0
            wbuf[m, WBLK["im_strip"] * P + m] = -1.0

        in_maps.append({"v_in": v_shard, "w_in": _tf32(wbuf.astype(np.float32))})
        masks.append(mask)
    return in_maps, masks


_LAST_RESULTS = None  # stashed BassKernelResults (for the test harness)


def kernel(V_norm: np.ndarray, S_grid: np.ndarray, t_grid: np.ndarray):
    global _PROGRAM, _LAST_RESULTS
    import os

    V = np.asarray(V_norm, dtype=np.float32).reshape(N_S, N_T)
    S = np.asarray(S_grid, dtype=np.float32).reshape(N_S)
    t = np.asarray(t_grid, dtype=np.float32).reshape(N_T)

    if _PROGRAM is None:
        _PROGRAM = _build_program()
    nc = _PROGRAM

    in_maps, masks = _host_inputs_and_masks(V, S)
    trace = bool(os.environ.get("BSLOSS_TRACE"))
    res = run_bass_kernel_spmd(nc, in_maps, core_ids=list(range(N_CORES)),
                               trace=trace)
    _LAST_RESULTS = res

    pde_sum = 0.0
    for c in range(N_CORES):
        stats = res.results[c]["stats_out"].astype(np.float64)
        pde_sum += float((masks[c].astype(np.float64) * stats).sum())
    n_int = (N_S - 2) * (N_T - 2)
    pde_loss = pde_sum / n_int

    # ---- boundary losses on host (tiny O(N) edge terms), float64 ----
    V64 = V.astype(np.float64)
    S64 = S.astype(np.float64)
    t64 = t.astype(np.float64)

    loss_S0 = float((V64[0, :] ** 2).sum() / N_T)

    tau = 1.0 - t64
    V_ff = 1.0 - K * np.exp(-R * tau) / SMAX
    loss_Smax = float(((V64[N_S - 1, :] - V_ff) ** 2).sum() / N_T)

    x = SOFTPLUS_BETA * (S64 - K / SMAX)
    payoff = (np.maximum(x, 0.0) + np.log1p(np.exp(-np.abs(x)))) / SOFTPLUS_BETA
    diff_T = V64[:, N_T - 1] - payoff
    abs_d = np.abs(diff_T)
    huber = np.where(abs_d < HUBER_DELTA, 0.5 * diff_T ** 2,
                     HUBER_DELTA * (abs_d - 0.5 * HUBER_DELTA))
    loss_T = float(huber.sum() / N_S)

    total = L_PDE * pde_loss + L_BC * loss_Smax + L_TC * loss_T
    return (np.float32(total), np.float32(pde_loss), np.float32(loss_S0),
            np.float32(loss_Smax), np.float32(loss_T))


# revision 2
# speedup vs baseline: 1.0938x; 1.0938x over previous
"""Trainium2 Bass kernel for nn_BSLoss (Black-Scholes PINN loss on a 4096x4096 grid).

Strategy (8 NeuronCores, SPMD, S-sharded):
  - Each core takes 512 grid rows (+1-row halos, host-sliced) x all 4096 t-columns.
  - PDE residual r/c = tri_S(V)/c + (V[:,j+1] - V[:,j-1]); the S-stencil is one
    [128x128] tridiagonal float32r (tf32) stationary matmul per 512-col chunk
    (contraction over partitions = cross-partition shifts for free). The t-stencil
    is either two +/-1-identity bf16 matmuls against column-shifted moving APs
    (PE groups) or a DVE tensor-sub + scalar_tensor_tensor accumulate into PSUM
    (DVE groups) - split to balance engines.
  - Columns are processed in 2 halves x 4 chunks; each (tile, half) group
    accumulates into a 4-bank [128, 2048] PSUM tile, squared+reduced by a single
    ScalarEngine activation(Square, accum_out) into per-row stats.
  - Host applies row masks (+c^2 scale) to the [128, 10] stats, sums in float64,
    and computes the O(N) boundary losses (rows 0/4095, col 4095).

Grid boundary columns (t=0, t=4095) are excluded by construction: chunks cover
columns 1..4094 only.
"""
import os
import sys

if "/opt/trn_rl_repo" not in sys.path:
    sys.path.insert(0, "/opt/trn_rl_repo")

import numpy as np
import ml_dtypes

import concourse.mybir as mybir
import concourse.tile as tile
from concourse import bacc
from concourse.bass_utils import run_bass_kernel_spmd

# ---- problem constants (match the reference) ----
N_S, N_T = 4096, 4096
R, SIGMA, K, T_MAT, SMAX = 0.05, 0.2, 100.0, 1.0, 300.0
B_STR, ALPHA = K / SMAX, 0.5
L_PDE, L_BC, L_TC = 1.0, 10.0, 10.0
HUBER_DELTA = 0.01
SOFTPLUS_BETA = 50.0

N_CORES = 8
ROWS_PER_CORE = N_S // N_CORES          # 512
IN_ROWS = ROWS_PER_CORE + 2             # 514 (with halos)
P = 128
TILE_STARTS = [0, 126, 252, 378]        # full tiles; outputs local rows 1..504
STRIP_START = 504                       # strip tile rows 504..513 -> outputs 505..512
STRIP_K = 10
N_TILES = 5
C_T = (N_T - 1) / 2.0 / T_MAT           # 2047.5

# column halves: half 0 = global cols [0, 2056); half 1 = [2040, 4096)
HALF_OFF = [0, 2040]
HALF_W = 2056
# chunks per (half): (local_start, width); global a = local + HALF_OFF
CHUNKS_H = [
    [(1, 512), (513, 512), (1025, 512), (1537, 512)],      # global 1..2049
    [(9, 512), (521, 512), (1033, 512), (1545, 510)],      # global 2049..4094
]
GROUP_W = [2048, 2046]
N_GROUPS = N_TILES * 2
# groups whose t-stencil runs on DVE (TT sub + STT accumulate into PSUM)
DVE_SHIFT = {(4, 0), (4, 1), (3, 1), (2, 1), (1, 1)}

F32 = mybir.dt.float32
F32R = mybir.dt.float32r
BF16 = mybir.dt.bfloat16


def _solve_cubic(Q: float) -> float:
    c = -Q
    for _ in range(5):
        f = c ** 3 / 6.0 + c + Q
        df = 0.5 * c * c + 1.0
        c = c - f / df
    return c


C1 = _solve_cubic((B_STR - 0.0) / ALPHA)
C2 = _solve_cubic((B_STR - 1.0) / ALPHA)


def _tf32(x: np.ndarray) -> np.ndarray:
    """Round float32 to tfloat32 (10-bit mantissa, round-to-nearest)."""
    u = np.ascontiguousarray(x, dtype=np.float32).view(np.uint32).astype(np.uint64)
    u = (u + np.uint64(0x1000)) & np.uint64(0xFFFFE000)
    return u.astype(np.uint32).view(np.float32)


def _stencil_coeffs(S: np.ndarray):
    """Per-row stencil coefficients / C_T (c folded out; re-applied via host mask)."""
    S = S.astype(np.float64)
    dS = 1.0 / (N_S - 1)
    L = C2 * S + C1 * (1.0 - S)
    dL = C2 - C1
    S_u = ALPHA * dL * (0.5 * L ** 2 + 1.0)
    S_uu = ALPHA * dL ** 2 * L
    e = 0.5 * SIGMA ** 2 * S ** 2
    f = R * S
    a_uu = e / S_u ** 2
    a_u = f / S_u - e * S_uu / S_u ** 3
    hi = a_uu / dS ** 2 + a_u / (2 * dS)
    lo = a_uu / dS ** 2 - a_u / (2 * dS)
    mid = -2.0 * a_uu / dS ** 2 - R
    return lo / C_T, mid / C_T, hi / C_T


_PROGRAM = None


def _build_program():
    nc = bacc.Bacc("TRN2", target_bir_lowering=False)

    v_in = nc.dram_tensor("v_in", [IN_ROWS, N_T], F32R, kind="ExternalInput")
    wtri_in = nc.dram_tensor("wtri_in", [P, N_TILES * P], F32R, kind="ExternalInput")
    wid_in = nc.dram_tensor("wid_in", [P, 2 * P], BF16, kind="ExternalInput")
    stats_out = nc.dram_tensor("stats_out", [P, N_GROUPS], F32, kind="ExternalOutput")

    with tile.TileContext(nc) as tc:
        with (
            tc.tile_pool(name="vpool", bufs=1) as vpool,
            tc.tile_pool(name="wpool", bufs=1) as wpool,
            tc.tile_pool(name="cpool", bufs=1) as cpool,
            tc.tile_pool(name="scratch", bufs=2) as spool,
            tc.tile_pool(name="psum", bufs=2, space="PSUM") as psum_pool,
        ):
            wtri = wpool.tile([P, N_TILES * P], F32R)
            nc.sync.dma_start(wtri[:], wtri_in[:])
            wid = wpool.tile([P, 2 * P], BF16)
            nc.sync.dma_start(wid[:], wid_in[:])
            stats = wpool.tile([P, N_GROUPS], F32)

            # half tiles, DMA'd in compute order
            vh = {}
            for t in range(N_TILES):
                kdim = P if t < 4 else STRIP_K
                r0 = TILE_STARTS[t] if t < 4 else STRIP_START
                for h in (0, 1):
                    vt = vpool.tile([kdim, HALF_W], F32R, tag=f"v{t}{h}")
                    nc.sync.dma_start(
                        vt[:], v_in[r0:r0 + kdim, HALF_OFF[h]:HALF_OFF[h] + HALF_W])
                    vh[(t, h)] = vt

            # bf16 casts for PE-shift groups
            vb = {}
            for t in range(N_TILES):
                for h in (0, 1):
                    if (t, h) in DVE_SHIFT:
                        continue
                    kdim = P if t < 4 else STRIP_K
                    vbt = cpool.tile([kdim, HALF_W], BF16, tag=f"vb{t}{h}")
                    nc.vector.tensor_copy(vbt[:], vh[(t, h)][:].bitcast(F32))
                    vb[(t, h)] = vbt

            for t in range(N_TILES):
                kdim = P if t < 4 else STRIP_K
                tri = wtri[0:kdim, t * P:(t + 1) * P]
                for h in (0, 1):
                    u = t * 2 + h
                    gw = GROUP_W[h]
                    chunks = CHUNKS_H[h]
                    v = vh[(t, h)]
                    ps = psum_pool.tile([P, 2048], F32, tag="ps")
                    dve_shift = (t, h) in DVE_SHIFT
                    for ci, (la, cw) in enumerate(chunks):
                        nc.tensor.matmul(ps[:, 512 * ci:512 * ci + cw],
                                         lhsT=tri, rhs=v[0:kdim, la:la + cw],
                                         start=True, stop=dve_shift)
                    if dve_shift:
                        la0 = chunks[0][0]
                        wsc = spool.tile([P, 2048], F32, tag="w")
                        nc.vector.tensor_tensor(
                            out=wsc[0:kdim, 0:gw],
                            in0=v[0:kdim, la0 + 1:la0 + 1 + gw].bitcast(F32),
                            in1=v[0:kdim, la0 - 1:la0 - 1 + gw].bitcast(F32),
                            op=mybir.AluOpType.subtract)
                        nc.vector.scalar_tensor_tensor(
                            out=ps[0:kdim, 0:gw], in0=wsc[0:kdim, 0:gw],
                            scalar=1.0, in1=ps[0:kdim, 0:gw],
                            op0=mybir.AluOpType.mult, op1=mybir.AluOpType.add)
                    else:
                        vbt = vb[(t, h)]
                        for sweep, (blk, sh) in enumerate([(0, 1), (1, -1)]):
                            ident = wid[0:kdim, blk * P:blk * P + P]
                            for ci, (la, cw) in enumerate(chunks):
                                nc.tensor.matmul(
                                    ps[:, 512 * ci:512 * ci + cw], lhsT=ident,
                                    rhs=vbt[0:kdim, la + sh:la + sh + cw],
                                    start=False, stop=(sweep == 1))
                    sq = spool.tile([P, 2048], F32, tag="sq")
                    nc.scalar.activation(sq[:, 0:gw], ps[:, 0:gw],
                                         mybir.ActivationFunctionType.Square,
                                         accum_out=stats[:, u:u + 1])

            nc.sync.dma_start(stats_out[:], stats[:])

    nc.compile()
    return nc


def _host_inputs_and_masks(V: np.ndarray, S: np.ndarray):
    lo, mid, hi = _stencil_coeffs(S)
    c2 = float(C_T) ** 2

    in_maps = []
    masks = []
    wid = np.zeros((P, 2 * P), np.float32)
    wid[:, 0:P] = np.eye(P)
    wid[:, P:2 * P] = -np.eye(P)
    wid = wid.astype(ml_dtypes.bfloat16)

    for c in range(N_CORES):
        rows = np.clip(np.arange(512 * c - 1, 512 * c + 513), 0, N_S - 1)
        v_shard = _tf32(V[rows, :])

        wtri = np.zeros((P, N_TILES * P), np.float64)
        mask = np.zeros((P, N_GROUPS), np.float32)
        for t in range(N_TILES):
            if t < 4:
                t0, m_lo, m_hi = TILE_STARTS[t], 1, 126
            else:
                t0, m_lo, m_hi = STRIP_START, 1, 8
            for m in range(m_lo, m_hi + 1):
                L = t0 + m
                g = 512 * c - 1 + L
                if not (1 <= g <= N_S - 2):
                    continue
                wtri[m - 1, t * P + m] = lo[g]
                wtri[m, t * P + m] = mid[g]
                wtri[m + 1, t * P + m] = hi[g]
                mask[m, t * 2:t * 2 + 2] = c2
        in_maps.append({"v_in": v_shard,
                        "wtri_in": _tf32(wtri.astype(np.float32)),
                        "wid_in": wid})
        masks.append(mask)
    return in_maps, masks


_LAST_RESULTS = None  # stashed BassKernelResults (for the test harness)


def kernel(V_norm: np.ndarray, S_grid: np.ndarray, t_grid: np.ndarray):
    global _PROGRAM, _LAST_RESULTS

    V = np.asarray(V_norm, dtype=np.float32).reshape(N_S, N_T)
    S = np.asarray(S_grid, dtype=np.float32).reshape(N_S)
    t = np.asarray(t_grid, dtype=np.float32).reshape(N_T)

    if _PROGRAM is None:
        _PROGRAM = _build_program()
    nc = _PROGRAM

    in_maps, masks = _host_inputs_and_masks(V, S)
    trace = bool(os.environ.get("BSLOSS_TRACE"))
    res = run_bass_kernel_spmd(nc, in_maps, core_ids=list(range(N_CORES)),
                               trace=trace)
    _LAST_RESULTS = res

    pde_sum = 0.0
    for c in range(N_CORES):
        stats = res.results[c]["stats_out"].astype(np.float64)
        pde_sum += float((masks[c].astype(np.float64) * stats).sum())
    n_int = (N_S - 2) * (N_T - 2)
    pde_loss = pde_sum / n_int

    # ---- boundary losses on host (tiny O(N) edge terms), float64 ----
    V64 = V.astype(np.float64)
    S64 = S.astype(np.float64)
    t64 = t.astype(np.float64)

    loss_S0 = float((V64[0, :] ** 2).sum() / N_T)

    tau = 1.0 - t64
    V_ff = 1.0 - K * np.exp(-R * tau) / SMAX
    loss_Smax = float(((V64[N_S - 1, :] - V_ff) ** 2).sum() / N_T)

    x = SOFTPLUS_BETA * (S64 - K / SMAX)
    payoff = (np.maximum(x, 0.0) + np.log1p(np.exp(-np.abs(x)))) / SOFTPLUS_BETA
    diff_T = V64[:, N_T - 1] - payoff
    abs_d = np.abs(diff_T)
    huber = np.where(abs_d < HUBER_DELTA, 0.5 * diff_T ** 2,
                     HUBER_DELTA * (abs_d - 0.5 * HUBER_DELTA))
    loss_T = float(huber.sum() / N_S)

    total = L_PDE * pde_loss + L_BC * loss_Smax + L_TC * loss_T
    return (np.float32(total), np.float32(pde_loss), np.float32(loss_S0),
            np.float32(loss_Smax), np.float32(loss_T))
